# revision 16
# baseline (speedup 1.0000x reference)
"""Trainium2 Bass kernel v2 for the DEFT Bishop-frame rod problem.

This environment has ~40us per-instruction dispatch overhead, so the design
minimizes instruction count: full-width ops, a Hillis-Steele doubling scan
for quaternion prefixes (7 levels over 128 edges), then one application of
the prefix rotations to u0.
"""
import sys

sys.path.insert(0, "/opt/trn_rl_repo")

import numpy as np
import concourse.bass as bass
import concourse.mybir as mybir
from concourse import tile
from concourse.bass_utils import run_bass_kernel_spmd

AF = mybir.ActivationFunctionType
ALU = mybir.AluOpType
DT = mybir.dt.float32

NCORES = 8
NV = 129
E = 128
P = 128
MAG_THR = float(np.float32(4.0 * (1.0 - (1.0 - 1e-6) ** 2) / (1.0 - 1e-6) ** 2))

_CACHE = {}


def build_nc(R, reps=1):
    W = R // P
    assert R % P == 0
    nc = bass.Bass()
    v = nc.vector
    sc = nc.scalar

    verts = nc.dram_tensor("verts", [R, NV, 3], DT, kind="ExternalInput")
    init_d = nc.dram_tensor("init_direct", [R, 3], DT, kind="ExternalInput")
    m_theta = nc.dram_tensor("m_theta", [R, E], DT, kind="ExternalInput")
    restL = nc.dram_tensor("restEdgeL", [R, E], DT, kind="ExternalInput")
    out = nc.dram_tensor("out", [R, E, 5, 3], DT, kind="ExternalOutput")
    kbd = nc.dram_tensor("kb_scratch", [R, 3, E], DT)
    bud = nc.dram_tensor("bu_scratch", [R, 5, E], DT)

    vr = verts[:].rearrange("(p w) n c -> p w n c", p=P)
    ir = init_d[:].rearrange("(p w) c -> p w c", p=P)
    tr = m_theta[:].rearrange("(p w) e -> p w e", p=P)
    lr = restL[:].rearrange("(p w) e -> p w e", p=P)
    outr = out[:].rearrange("(p w) e f c -> p w e f c", p=P)
    kbr = kbd[:].rearrange("(p w) c e -> p w c e", p=P)
    bur = bud[:].rearrange("(p w) c e -> p w c e", p=P)

    with tile.TileContext(nc) as tc:
     for _rep in range(reps):
      with tc.tile_pool(name="res", bufs=1) as res:
        c4 = res.tile([P, 1], DT)
        v.memset(c4[:], 4.0)
        chpi = res.tile([P, 1], DT)
        v.memset(chpi[:], float(np.pi / 2))
        c0 = res.tile([P, 1], DT)
        v.memset(c0[:], 0.0)
        u0 = res.tile([P, W, 5], DT)

        with tc.tile_pool(name="qa", bufs=1) as qa:
            qA = qa.tile([P, W, 4, E], DT)       # quaternion planes w,x,y,z

            # ---------- construction + u0 -----------------------------------
            with tc.tile_pool(name="ce", bufs=1) as ce:
                e5 = ce.tile([P, W, 5, E], DT)          # edges + dup planes x,y
                with tc.tile_pool(name="cv", bufs=1) as cv:
                    vf = cv.tile([P, W, NV, 3], DT)
                    nc.sync.dma_start(vf[:], vr[:])
                    for cc in range(3):
                        v.tensor_tensor(out=e5[:, :, cc, :], in0=vf[:, :, 1:, cc],
                                        in1=vf[:, :, :-1, cc], op=ALU.subtract)
                    v.tensor_copy(out=e5[:, :, 3:5, :], in_=e5[:, :, 0:2, :])

                with tc.tile_pool(name="cw", bufs=1) as cw:
                    Lf = cw.tile([P, W, E], DT)
                    nc.sync.dma_start(Lf[:], lr[:])
                    for (lo, hi) in ((1, 65), (65, 128)):
                        n = hi - lo
                        kbt = cw.tile([P, W, 3, 64], DT, tag="kbt", name="kbt")
                        kbch = kbt[:, :, :, 0:n]
                        ep = lambda i: e5[:, :, i : i + 3, lo - 1 : hi - 1]
                        en = lambda i: e5[:, :, i : i + 3, lo:hi]
                        cr = cw.tile([P, W, 3, 64], DT, tag="cr", name="cr")[:, :, :, 0:n]
                        tp = cw.tile([P, W, 3, 64], DT, tag="tp", name="tp")[:, :, :, 0:n]
                        v.tensor_tensor(out=cr, in0=ep(1), in1=en(2), op=ALU.mult)
                        v.tensor_tensor(out=tp, in0=ep(2), in1=en(1), op=ALU.mult)
                        v.tensor_tensor(out=cr, in0=cr, in1=tp, op=ALU.subtract)
                        v.tensor_tensor(out=tp, in0=ep(0), in1=en(0), op=ALU.mult)
                        dd = cw.tile([P, W, 64], DT, tag="dd", name="dd")[:, :, 0:n]
                        v.tensor_reduce(out=dd, in_=tp.rearrange("p w c n -> p w n c"),
                                        axis=mybir.AxisListType.X, op=ALU.add)
                        den = cw.tile([P, W, 64], DT, tag="den", name="den")[:, :, 0:n]
                        v.tensor_tensor(out=den, in0=Lf[:, :, lo - 1 : hi - 1],
                                        in1=Lf[:, :, lo:hi], op=ALU.mult)
                        v.tensor_tensor(out=den, in0=den, in1=dd, op=ALU.add)
                        v.reciprocal(out=den, in_=den)
                        v.tensor_scalar_mul(den, den, 2.0)
                        dnb = den.unsqueeze(2).to_broadcast([P, W, 3, n])
                        v.tensor_tensor(out=kbch, in0=cr, in1=dnb, op=ALU.mult)
                        # mag & quaternion
                        v.tensor_tensor(out=tp, in0=kbch, in1=kbch, op=ALU.mult)
                        v.tensor_reduce(out=dd, in_=tp.rearrange("p w c n -> p w n c"),
                                        axis=mybir.AxisListType.X, op=ALU.add)
                        sc.activation(den, dd, AF.Sqrt, bias=c4[:])
                        v.reciprocal(out=den, in_=den)        # rsq
                        g = cw.tile([P, W, 64], DT, tag="g", name="g")[:, :, 0:n]
                        v.tensor_scalar(g, dd, MAG_THR, None, op0=ALU.is_gt)
                        v.tensor_tensor(out=den, in0=den, in1=g, op=ALU.mult)  # fg
                        fgb = den.unsqueeze(2).to_broadcast([P, W, 3, n])
                        v.tensor_tensor(out=qA[:, :, 1:4, lo:hi], in0=kbch, in1=fgb,
                                        op=ALU.mult)
                        v.tensor_scalar(dd, den, 2.0, 1.0, op0=ALU.mult, op1=ALU.add)
                        v.scalar_tensor_tensor(out=qA[:, :, 0, lo:hi], in0=g, scalar=-1.0,
                                               in1=dd, op0=ALU.mult, op1=ALU.add)
                        nc.sync.dma_start(kbr[:, :, :, lo:hi], kbch)
                    # edge 0: identity quaternion
                    v.memset(qA[:, :, 0, 0:1], 1.0)
                    v.memset(qA[:, :, 1:4, 0:1], 0.0)

                    # ---------- u0 ------------------------------------------
                    d5 = cw.tile([P, W, 5], DT, tag="d5")
                    nc.sync.dma_start(d5[:, :, 0:3], ir[:, :, :])
                    v.tensor_copy(out=d5[:, :, 3:5], in_=d5[:, :, 0:2])
                    n5 = cw.tile([P, W, 5], DT, tag="n5")
                    t3 = cw.tile([P, W, 3], DT, tag="t3")
                    s3 = cw.tile([P, W, 3], DT, tag="s3")
                    e05 = e5[:, :, :, 0]        # (P,W,5) first edge w/ dups
                    v.tensor_tensor(out=t3[:], in0=e05[:, :, 1:4], in1=d5[:, :, 2:5], op=ALU.mult)
                    v.tensor_tensor(out=s3[:], in0=e05[:, :, 2:5], in1=d5[:, :, 1:4], op=ALU.mult)
                    v.tensor_tensor(out=n5[:, :, 0:3], in0=t3[:], in1=s3[:], op=ALU.subtract)
                    v.tensor_copy(out=n5[:, :, 3:5], in_=n5[:, :, 0:2])
                    v.tensor_tensor(out=t3[:], in0=n5[:, :, 1:4], in1=e05[:, :, 2:5], op=ALU.mult)
                    v.tensor_tensor(out=s3[:], in0=n5[:, :, 2:5], in1=e05[:, :, 1:4], op=ALU.mult)
                    v.tensor_tensor(out=t3[:], in0=t3[:], in1=s3[:], op=ALU.subtract)
                    v.tensor_tensor(out=s3[:], in0=t3[:], in1=t3[:], op=ALU.mult)
                    nn = cw.tile([P, W], DT, tag="nn")
                    v.tensor_reduce(out=nn[:], in_=s3[:], axis=mybir.AxisListType.X, op=ALU.add)
                    sc.activation(nn[:], nn[:], AF.Sqrt, bias=c0[:])
                    v.reciprocal(out=nn[:], in_=nn[:])
                    nb = nn[:].unsqueeze(2).to_broadcast([P, W, 3])
                    v.tensor_tensor(out=u0[:, :, 0:3], in0=t3[:], in1=nb, op=ALU.mult)
                    v.tensor_copy(out=u0[:, :, 3:5], in_=u0[:, :, 0:2])

            # ---------- Hillis-Steele doubling scan (7 levels) --------------
            # P_i <- P_i o P_{i-h}; after 7 levels: inclusive prefixes.
            with tc.tile_pool(name="qs", bufs=1) as qs:
                qB = qs.tile([P, W, 4, E], DT)
                with tc.tile_pool(name="ab", bufs=1) as ab:
                    A = ab.tile([P, W, 4, E], DT)
                    t1 = ab.tile([P, W, 4, E], DT, tag="t1")
                    src, dst = qA, qB
                    for k in range(7):
                        h = 1 << k
                        n = E - h
                        v.tensor_copy(out=dst[:, :, :, 0:h], in_=src[:, :, :, 0:h])
                        Ah = A[:, :, :, h:E]
                        t1h = t1[:, :, :, h:E]
                        m0 = dst[:, :, :, h:E]
                        bsl = lambda kc: src[:, :, kc, 0 : E - h].unsqueeze(2).to_broadcast([P, W, 4, n])
                        v.tensor_tensor(out=m0, in0=src[:, :, :, h:E], in1=bsl(0), op=ALU.mult)
                        # A1 = (-x, w, z, -y)
                        v.tensor_copy(out=A[:, :, 1:3, h:E], in_=src[:, :, 0:4:3, h:E])
                        v.tensor_scalar_mul(A[:, :, 0:4:3, h:E], src[:, :, 1:3, h:E], -1.0)
                        v.tensor_tensor(out=t1h, in0=Ah, in1=bsl(1), op=ALU.mult)
                        v.tensor_tensor(out=m0, in0=m0, in1=t1h, op=ALU.add)
                        # A2 = (-y, -z, w, x)
                        v.tensor_copy(out=A[:, :, 2:4, h:E], in_=src[:, :, 0:2, h:E])
                        v.tensor_scalar_mul(A[:, :, 0:2, h:E], src[:, :, 2:4, h:E], -1.0)
                        v.tensor_tensor(out=t1h, in0=Ah, in1=bsl(2), op=ALU.mult)
                        v.tensor_tensor(out=m0, in0=m0, in1=t1h, op=ALU.add)
                        # A3 = (-z, y, -x, w)
                        v.tensor_copy(out=A[:, :, 1, h:E], in_=src[:, :, 2, h:E])
                        v.tensor_copy(out=A[:, :, 3, h:E], in_=src[:, :, 0, h:E])
                        v.tensor_scalar_mul(A[:, :, 0, h:E], src[:, :, 3, h:E], -1.0)
                        v.tensor_scalar_mul(A[:, :, 2, h:E], src[:, :, 1, h:E], -1.0)
                        v.tensor_tensor(out=t1h, in0=Ah, in1=bsl(3), op=ALU.mult)
                        v.tensor_tensor(out=m0, in0=m0, in1=t1h, op=ALU.add)
                        src, dst = dst, src
                    # 7 levels end with result in qB; move to qA so qB frees
                    v.tensor_copy(out=qA[:], in_=qB[:])
            Q = qA

            # ---------- apply + post ----------------------------------------
            with tc.tile_pool(name="bup", bufs=1) as bup:
                bu = bup.tile([P, W, 5, E], DT)
                for (lo, hi) in ((0, 64), (64, E)):
                    n = hi - lo
                    uv = bup.tile([P, W, 3, 64], DT, tag="uv", name="uv")[:, :, :, 0:n]
                    kk = bup.tile([P, W, 3, 64], DT, tag="kk", name="kk")[:, :, :, 0:n]
                    mm = bup.tile([P, W, 3, 64], DT, tag="mm", name="mm")[:, :, :, 0:n]
                    u0c = lambda i: u0[:, :, i].unsqueeze(2).unsqueeze(3).to_broadcast([P, W, 1, n])
                    Qp = lambda i: Q[:, :, i : i + 1, lo:hi]
                    # uv = Qv x u0 (component-wise; u0 comps broadcast)
                    for c in range(3):
                        a, b = (c + 1) % 3, (c + 2) % 3
                        v.tensor_tensor(out=mm[:, :, c : c + 1, :], in0=Qp(1 + a), in1=u0c(b), op=ALU.mult)
                        v.tensor_tensor(out=kk[:, :, c : c + 1, :], in0=Qp(1 + b), in1=u0c(a), op=ALU.mult)
                    v.tensor_tensor(out=uv, in0=mm, in1=kk, op=ALU.subtract)
                    # kk = Qv x uv (component-wise)
                    for c in range(3):
                        a, b = (c + 1) % 3, (c + 2) % 3
                        v.tensor_tensor(out=mm[:, :, c : c + 1, :], in0=Qp(1 + a),
                                        in1=uv[:, :, b : b + 1, :], op=ALU.mult)
                        v.tensor_tensor(out=kk[:, :, c : c + 1, :], in0=Qp(1 + b),
                                        in1=uv[:, :, a : a + 1, :], op=ALU.mult)
                    v.tensor_tensor(out=kk, in0=mm, in1=kk, op=ALU.subtract)
                    qwb = Q[:, :, 0, lo:hi].unsqueeze(2).to_broadcast([P, W, 3, n])
                    v.tensor_tensor(out=mm, in0=qwb, in1=uv, op=ALU.mult)
                    v.tensor_tensor(out=mm, in0=mm, in1=kk, op=ALU.add)
                    v.tensor_scalar_mul(mm, mm, 2.0)
                    u03b = u0[:, :, 0:3].unsqueeze(3).to_broadcast([P, W, 3, n])
                    v.tensor_tensor(out=bu[:, :, 0:3, lo:hi], in0=mm, in1=u03b, op=ALU.add)
                    v.tensor_copy(out=bu[:, :, 3:5, lo:hi], in_=bu[:, :, 0:2, lo:hi])
                nc.sync.dma_start(bur[:], bu[:])

        # ---------- post (4 chunks of 32 edges), qa/bup pools closed --------
        if True:
            if True:
                with tc.tile_pool(name="pk", bufs=1) as pk:
                    kbb = pk.tile([P, W, 3, E], DT)
                    nc.sync.dma_start(kbb[:, :, :, 1:E], kbr[:, :, :, 1:E])
                    v.memset(kbb[:, :, :, 0:1], 0.0)
                    bu = pk.tile([P, W, 5, E], DT, tag="bu2", name="bu2")
                    nc.sync.dma_start(bu[:], bur[:])
                    with tc.tile_pool(name="pw", bufs=1) as pw, \
                         tc.tile_pool(name="stgp", bufs=1) as stgp:
                        for ci in range(4):
                            lo, hi = ci * 32, ci * 32 + 32
                            n = 32
                            vp = pw.tile([P, W, n + 1, 3], DT, tag="vp")
                            nc.sync.dma_start(vp[:], vr[:, :, lo : hi + 1, :])
                            th = pw.tile([P, W, n], DT, tag="th")
                            nc.sync.dma_start(th[:], tr[:, :, lo:hi])
                            ep5 = pw.tile([P, W, 5, n], DT, tag="ep5")
                            for cc in range(3):
                                v.tensor_tensor(out=ep5[:, :, cc, :], in0=vp[:, :, 1:, cc],
                                                in1=vp[:, :, :-1, cc], op=ALU.subtract)
                            v.tensor_copy(out=ep5[:, :, 3:5, :], in_=ep5[:, :, 0:2, :])
                            ub = bu[:, :, :, lo:hi]
                            w1 = pw.tile([P, W, 3, n], DT, tag="w1")
                            w2 = pw.tile([P, W, 3, n], DT, tag="w2")
                            v.tensor_tensor(out=w1[:], in0=ep5[:, :, 1:4, :], in1=ub[:, :, 2:5, :], op=ALU.mult)
                            v.tensor_tensor(out=w2[:], in0=ep5[:, :, 2:5, :], in1=ub[:, :, 1:4, :], op=ALU.mult)
                            v.tensor_tensor(out=w1[:], in0=w1[:], in1=w2[:], op=ALU.subtract)
                            v.tensor_tensor(out=w2[:], in0=w1[:], in1=w1[:], op=ALU.mult)
                            bm = pw.tile([P, W, n], DT, tag="bm")
                            v.tensor_reduce(out=bm[:], in_=w2[:].rearrange("p w c n -> p w n c"),
                                            axis=mybir.AxisListType.X, op=ALU.add)
                            sc.activation(bm[:], bm[:], AF.Sqrt, bias=c0[:])
                            v.reciprocal(out=bm[:], in_=bm[:])
                            stg = stgp.tile([P, W, n, 15], DT, tag="stg")
                            v.tensor_copy(out=stg[:, :, :, 0:3],
                                          in_=ub[:, :, 0:3, :].rearrange("p w c n -> p w n c"))
                            rbb = bm[:].unsqueeze(3).to_broadcast([P, W, n, 3])
                            v.tensor_tensor(out=stg[:, :, :, 3:6],
                                            in0=w1[:].rearrange("p w c n -> p w n c"),
                                            in1=rbb, op=ALU.mult)
                            v.tensor_copy(out=stg[:, :, :, 6:9],
                                          in_=kbb[:, :, :, lo:hi].rearrange("p w c n -> p w n c"))
                            cosq = pw.tile([P, W, n], DT, tag="cosq")
                            sc.activation(cosq[:], th[:], AF.Sin, bias=chpi[:])
                            sinq = pw.tile([P, W, n], DT, tag="sinq")
                            sc.activation(sinq[:], th[:], AF.Sin, bias=c0[:])
                            cb = cosq[:].unsqueeze(3).to_broadcast([P, W, n, 3])
                            sb = sinq[:].unsqueeze(3).to_broadcast([P, W, n, 3])
                            t1p = pw.tile([P, W, n, 3], DT, tag="w1", name="t1p")
                            t2p = pw.tile([P, W, n, 3], DT, tag="w2", name="t2p")
                            v.tensor_tensor(out=t1p[:], in0=cb, in1=stg[:, :, :, 0:3], op=ALU.mult)
                            v.tensor_tensor(out=t2p[:], in0=sb, in1=stg[:, :, :, 3:6], op=ALU.mult)
                            v.tensor_tensor(out=stg[:, :, :, 9:12], in0=t1p[:], in1=t2p[:], op=ALU.add)
                            v.tensor_tensor(out=t1p[:], in0=cb, in1=stg[:, :, :, 3:6], op=ALU.mult)
                            v.tensor_tensor(out=t2p[:], in0=sb, in1=stg[:, :, :, 0:3], op=ALU.mult)
                            v.tensor_tensor(out=stg[:, :, :, 12:15], in0=t1p[:], in1=t2p[:], op=ALU.subtract)
                            nc.sync.dma_start(outr[:, :, lo:hi, :, :], stg[:])

    return nc


def _split_excess_waits(nc):
    """This walrus build encodes at most 1 sync wait per instruction; move
    excess waits onto NoOp carriers inserted just before, same engine."""
    MAXW = 1
    for func in nc.m.functions:
        for bb in func.blocks:
            insts = bb.instructions
            new_list = []
            changed = False
            for inst in insts:
                si = inst.sync_info
                waits = list(si.on_wait) if si is not None and si.on_wait else []
                if len(waits) > MAXW:
                    excess = waits[:-MAXW]
                    for j in range(0, len(excess), MAXW):
                        nop = mybir.InstNoOp(name=f"waitfix-{nc.next_id()}",
                                             engine=inst.engine)
                        nop.sync_info = mybir.SyncInfo(
                            on_wait=excess[j : j + MAXW], on_update=[])
                        new_list.append(nop)
                    si.on_wait = waits[-MAXW:]
                    changed = True
                new_list.append(inst)
            if changed:
                try:
                    bb.instructions = new_list
                except Exception:
                    insts.clear()
                    insts.extend(new_list)


def kernel(**inputs):
    verts = np.ascontiguousarray(inputs["verts"], dtype=np.float32)
    init_d = np.ascontiguousarray(inputs["init_direct"], dtype=np.float32)
    m_theta = np.ascontiguousarray(inputs["m_theta"], dtype=np.float32)
    restL = np.ascontiguousarray(inputs["restEdgeL"], dtype=np.float32)
    B = verts.shape[0]
    R = B // NCORES
    if "nc" not in _CACHE or _CACHE.get("R") != R:
        nc_new = build_nc(R)
        _split_excess_waits(nc_new)
        _CACHE["nc"] = nc_new
        _CACHE["R"] = R
    nc = _CACHE["nc"]
    in_maps = []
    for i in range(NCORES):
        sl = slice(i * R, (i + 1) * R)
        in_maps.append({
            "verts": verts[sl],
            "init_direct": init_d[sl],
            "m_theta": m_theta[sl],
            "restEdgeL": restL[sl],
        })
    res = run_bass_kernel_spmd(nc, in_maps, core_ids=list(range(NCORES)))
    return np.concatenate([res.results[i]["out"] for i in range(NCORES)], axis=0)
